# revision 16
# baseline (speedup 1.0000x reference)
# Trainium2 Bass kernel for KNN-style sparse cross-attention (v2).
#
# reference semantics:
#   q  = src @ w_src.T                           [B,S,D]
#   kv = tgt @ w_tgt.T                           [B,S,T,2D]
#   attn[b,h,s,t] = <q[b,s,h], k[b,s,t,h]> / sqrt(dh)  (per-query keys)
#   softmax over t (padding mask; fully-masked queries output 0)
#   out = (attn @ v) @ out_proj.T
#
# v2 strategy (vs v1):
#  - shard B*S = 2048 queries across 8 cores (256 queries, 8192 kv rows each)
#  - k-projection runs in fp8-e4m3 DoubleRow matmuls (2x PE throughput);
#    v-projection stays fp16 (v errors pass straight to the output).
#    k is scaled by SX*SW on chip; the descale rides the softmax exp scale.
#  - "dh-major" feature shuffle: k/q/v features are permuted so that within
#    each 128-partition tile, partition p belongs to head p%8.  The score
#    one-hot (em) and the attn broadcast (fm) then become j-independent:
#    one stationary serves all 4 feature tiles, and the attn->128-partition
#    broadcast is a single 8-partition matmul per chunk.
#  - scores for 4 chunks are packed into one PSUM bank at partition offsets
#    {0,32,64,96} via matmul tile_position, so softmax (exp/mask/sum/recip/
#    normalize) runs on 128 full partitions instead of 8.
#  - engine assignment: PE kv-proj+scores+broadcast, DVE q*k multiply (from
#    PSUM) + softmax + t-reduce, Scalar v/bc copies + exp, GPSIMD attn*v.
import os
from contextlib import ExitStack

import numpy as np
import ml_dtypes

import concourse.bacc as bacc
import concourse.mybir as mybir
import concourse.tile as tile
from concourse import bass_utils

N_CORES = 8
D = 512          # d_model
H = 8            # heads
DH = 64          # head dim
T = 32           # KNN set size per query
BS = 2048        # B*S total queries
R = BS // N_CORES     # 256 queries per core
RT = R * T            # 8192 kv rows per core
PT = 128              # partition tile
KD = D // PT          # 4 feature tiles
W = 512               # kv cols per superchunk
NSC = RT // W         # 16 superchunks
QS = W // T           # 16 queries per superchunk
SX = 16.0             # fp8 scale on tgt
SW = 512.0            # fp8 scale on w_k

F32 = mybir.dt.float32
F16 = mybir.dt.float16
F8 = mybir.dt.float8e4
AX = mybir.AxisListType
ALU = mybir.AluOpType
ACTF = mybir.ActivationFunctionType
PM = mybir.MatmulPerfMode


def build_program(n_cores=N_CORES):
    nc = bacc.Bacc(
        "TRN2",
        target_bir_lowering=False,
        debug=False,
        enable_asserts=False,
        num_devices=n_cores,
    )

    # All weight/const tensors are pre-arranged host-side into their exact
    # on-chip [128, free] layouts.
    srcT = nc.dram_tensor("srcT", [PT, KD * R], F16, kind="ExternalInput").ap()
    tg8 = nc.dram_tensor("tg8", [D, RT], F8, kind="ExternalInput").ap()
    tg16 = nc.dram_tensor("tg16", [D, RT], F16, kind="ExternalInput").ap()
    wsT = nc.dram_tensor("wsT", [PT, KD * D], F16, kind="ExternalInput").ap()
    wtk = nc.dram_tensor("wtk", [PT, 4 * D], F8, kind="ExternalInput").ap()
    wtv = nc.dram_tensor("wtv", [PT, KD * D], F16, kind="ExternalInput").ap()
    woT = nc.dram_tensor("woT", [PT, KD * D], F16, kind="ExternalInput").ap()
    em64 = nc.dram_tensor("em64", [PT, 64], F16, kind="ExternalInput").ap()
    bhun = nc.dram_tensor("bhun", [1, 64], F16, kind="ExternalInput").ap()
    fmrep = nc.dram_tensor("fmrep", [PT, PT], F16, kind="ExternalInput").ap()
    bmask = nc.dram_tensor("bmask", [1, RT], F16, kind="ExternalInput").ap()
    zmask = nc.dram_tensor("zmask", [PT, (NSC // 2) * QS], F32, kind="ExternalInput").ap()
    outT = nc.dram_tensor("outT", [D, R], F32, kind="ExternalOutput").ap()

    lp = nc.allow_low_precision("fp16 stores of fp32 internal math")
    lp.__enter__()
    with tile.TileContext(nc) as tc, ExitStack() as ctx:
        consts = ctx.enter_context(tc.tile_pool(name="consts", bufs=1))
        tg8p = ctx.enter_context(tc.tile_pool(name="tg8p", bufs=2))
        tg16p = ctx.enter_context(tc.tile_pool(name="tg16p", bufs=2))
        pjp = ctx.enter_context(tc.tile_pool(name="pjp", bufs=3))
        v16p = ctx.enter_context(tc.tile_pool(name="v16p", bufs=7))
        bc16p = ctx.enter_context(tc.tile_pool(name="bc16p", bufs=3))
        utp = ctx.enter_context(tc.tile_pool(name="utp", bufs=3))
        work = ctx.enter_context(tc.tile_pool(name="work", bufs=2))
        one = ctx.enter_context(tc.tile_pool(name="one", bufs=1))
        kvps = ctx.enter_context(tc.tile_pool(name="kvps", bufs=2, space="PSUM"))
        spp = ctx.enter_context(tc.tile_pool(name="spp", bufs=2, space="PSUM"))
        bcps = ctx.enter_context(tc.tile_pool(name="bcps", bufs=2, space="PSUM"))

        # ---- consts (critical-path order: qproj needs ws+src, first k-mms
        # need wtk + tg8(0); everything else can trail) ----
        ws_sb = consts.tile([PT, KD * D], F16, name="ws_sb")
        nc.sync.dma_start(ws_sb, wsT)
        src_sb = consts.tile([PT, KD * R], F16, name="src_sb")
        nc.sync.dma_start(src_sb, srcT)
        wtk_sb = consts.tile([PT, 4 * D], F8, name="wtk_sb")
        nc.sync.dma_start(wtk_sb, wtk)
        em_sb = consts.tile([PT, 64], F16, name="em_sb")
        nc.sync.dma_start(em_sb, em64)
        wtv_sb = consts.tile([PT, KD * D], F16, name="wtv_sb")
        nc.sync.dma_start(wtv_sb, wtv)

        qT = one.tile([PT, KD * R], F16, name="qT")
        oav = one.tile([PT, KD * R], F16, name="oav")

        tg8d = tg8.rearrange("(j p) n -> p j n", p=PT)
        tg16d = tg16.rearrange("(j p) n -> p j n", p=PT)

        def load_tg(sc):
            t8 = tg8p.tile([PT, KD * W], F8, name="t8")
            nc.sync.dma_start(
                t8.rearrange("p (j n) -> p j n", j=KD),
                tg8d[:, :, sc * W : (sc + 1) * W],
            )
            t16 = tg16p.tile([PT, KD * W], F16, name="t16")
            nc.sync.dma_start(
                t16.rearrange("p (j n) -> p j n", j=KD),
                tg16d[:, :, sc * W : (sc + 1) * W],
            )
            return t8, t16

        def qproj():
            for e in range(KD):
                qp = bcps.tile([PT, R], F32, name="qp", tag="bcp")
                for j in range(KD):
                    nc.tensor.matmul(
                        qp,
                        ws_sb[:, j * D + e * PT : j * D + (e + 1) * PT],
                        src_sb[:, j * R : (j + 1) * R],
                        start=(j == 0),
                        stop=(j == KD - 1),
                    )
                nc.scalar.copy(qT[:, e * R : (e + 1) * R], qp)

        # non-critical consts (after tg(0) is queued below)
        fm_sb = consts.tile([PT, PT], F16, name="fm_sb")
        bm_sb = consts.tile([1, RT], F16, name="bm_sb")
        bh_sb = consts.tile([1, 64], F16, name="bh_sb")
        wo_sb = consts.tile([PT, KD * D], F16, name="wo_sb")
        zm_sb = consts.tile([PT, (NSC // 2) * QS], F32, name="zm_sb")

        def late_consts():
            nc.sync.dma_start(fm_sb, fmrep)
            nc.sync.dma_start(bm_sb, bmask)
            nc.sync.dma_start(bh_sb, bhun)
            nc.sync.dma_start(wo_sb, woT)
            nc.sync.dma_start(zm_sb, zmask)

        # per-stage state, keyed by sc
        st = {}

        def k_mms(sc, t8):
            """fp8 DoubleRow k-projection: two [128, 2*W] psum tiles."""
            t8r = t8.rearrange("p (j n) -> p j n", j=KD)
            wk = wtk_sb.rearrange("p (g jj m) -> p g jj m", g=2, jj=2)
            kts = []
            for half in range(2):
                kt = kvps.tile([PT, 2 * W], F32, name="kt", tag="kv")
                for e01 in range(2):
                    e = 2 * half + e01
                    for g in range(2):
                        nc.tensor.matmul(
                            kt[:, e01 * W : (e01 + 1) * W],
                            wk[:, g, :, e * PT : (e + 1) * PT],
                            t8r[:, 2 * g : 2 * g + 2, :],
                            start=(g == 0),
                            stop=(g == 1),
                            perf_mode=PM.DoubleRow,
                        )
                kts.append(kt)
            return kts

        def pmul(sc, kts):
            """pj = k * q (broadcast over t), DVE reading k from PSUM."""
            pj = pjp.tile([PT, KD * W], F16, name="pj")
            for half, kt in enumerate(kts):
                nc.vector.tensor_mul(
                    pj.rearrange("p (e q t) -> p e q t", e=KD, t=T)[
                        :, 2 * half : 2 * half + 2
                    ],
                    kt.rearrange("p (e q t) -> p e q t", e=2, t=T),
                    qT.rearrange("p (e r) -> p e r", e=KD)[
                        :, 2 * half : 2 * half + 2, sc * QS : (sc + 1) * QS
                    ]
                    .unsqueeze(3)
                    .broadcast_to([PT, 2, QS, T]),
                )
            return pj

        def smm(sc, pj, spss):
            c = sc % 2
            nc.tensor.matmul(
                spss[64 * c : 64 * c + 64, :],
                bh_sb,
                bm_sb[:, sc * W : (sc + 1) * W],
                start=True,
                stop=False,
            )
            for jf in range(KD):
                nc.tensor.matmul(
                    spss[64 * c : 64 * c + 64, :],
                    em_sb,
                    pj[:, jf * W : (jf + 1) * W],
                    start=False,
                    stop=(jf == KD - 1),
                )

        def v_mms(sc, t16):
            t16r = t16.rearrange("p (j n) -> p j n", j=KD)
            vts = []
            for half in range(2):
                vt = kvps.tile([PT, 2 * W], F32, name="vt", tag="kv")
                for e01 in range(2):
                    e = 2 * half + e01
                    for j in range(KD):
                        nc.tensor.matmul(
                            vt[:, e01 * W : (e01 + 1) * W],
                            wtv_sb[:, j * D + e * PT : j * D + (e + 1) * PT],
                            t16r[:, j, :],
                            start=(j == 0),
                            stop=(j == KD - 1),
                        )
                vts.append(vt)
            return vts

        def v_copies(sc, vts):
            v16 = v16p.tile([PT, KD * W], F16, name="v16")
            for half, vt in enumerate(vts):
                nc.scalar.copy(v16[:, 2 * half * W : (2 * half + 2) * W], vt)
            return v16

        def softmax_group(g2, spss):
            exf = work.tile([PT, W], F16, name="exf")
            nc.scalar.activation(exf, spss, ACTF.Exp, scale=1.0 / (SX * SW))
            sums = work.tile([PT, QS], F32, name="sums")
            nc.vector.reduce_sum(
                sums, exf.rearrange("p (q t) -> p q t", t=T), axis=AX.X
            )
            rec = work.tile([PT, QS], F32, name="rec")
            nc.vector.reciprocal(rec, sums)
            recz = work.tile([PT, QS], F32, name="recz")
            nc.vector.tensor_mul(recz, rec, zm_sb[:, g2 * QS : (g2 + 1) * QS])
            attn = work.tile([PT, W], F16, name="attn")
            nc.gpsimd.tensor_mul(
                attn.rearrange("p (q t) -> p q t", t=T),
                exf.rearrange("p (q t) -> p q t", t=T),
                recz.unsqueeze(2).broadcast_to([PT, QS, T]),
            )
            return attn

        def bc_stage(sc, attn):
            c = sc % 2
            bcp = bcps.tile([PT, W], F32, name="bcp", tag="bcp")
            nc.tensor.matmul(
                bcp,
                fm_sb[64 * c : 64 * c + 8, :],
                attn[64 * c : 64 * c + 8, :],
                start=True,
                stop=True,
            )
            bc16 = bc16p.tile([PT, W], F16, name="bc16")
            nc.scalar.copy(bc16, bcp)
            return bc16

        def ut_stage(sc, v16, bc16):
            ut = utp.tile([PT, KD * W], F16, name="ut")
            nc.vector.tensor_mul(
                ut.rearrange("p (j n) -> p j n", j=KD),
                v16.rearrange("p (j n) -> p j n", j=KD),
                bc16.unsqueeze(1).broadcast_to([PT, KD, W]),
            )
            return ut

        def red_stage(sc, ut):
            nc.vector.reduce_sum(
                oav.rearrange("p (j r) -> p j r", j=KD)[
                    :, :, sc * QS : (sc + 1) * QS
                ],
                ut.rearrange("p (j q t) -> p j q t", j=KD, t=T),
                axis=AX.X,
            )

        # ---- software pipeline ----
        t8, t16 = load_tg(0)
        st[0] = dict(t8=t8, t16=t16)
        qproj()
        late_consts()

        def outproj_half(h2):
            q0, q1 = h2 * (R // 2), (h2 + 1) * (R // 2)
            for e in range(KD):
                op = bcps.tile([PT, R // 2], F32, name="op", tag="bcp")
                for j in range(KD):
                    nc.tensor.matmul(
                        op,
                        wo_sb[:, j * D + e * PT : j * D + (e + 1) * PT],
                        oav[:, j * R + q0 : j * R + q1],
                        start=(j == 0),
                        stop=(j == KD - 1),
                    )
                res = work.tile([PT, R // 2], F32, name="res")
                nc.scalar.copy(res, op)
                nc.sync.dma_start(outT[e * PT : (e + 1) * PT, q0:q1], res)

        LAG_BC = 3   # bc/ut for sc-3
        LAG_RD = 3   # reduce right after ut (same engine, in order)
        for it in range(NSC + LAG_RD + 1):
            sc = it
            if sc < NSC:
                if sc + 1 < NSC:
                    t8n, t16n = load_tg(sc + 1)
                    st[sc + 1] = dict(t8=t8n, t16=t16n)
                s = st[sc]
                s["kts"] = k_mms(sc, s["t8"])
                s["pj"] = pmul(sc, s["kts"])
            # smm for sc-1 (k/pj of sc-1 are done; PE does not stall on DVE)
            pv = sc - 1
            if 0 <= pv < NSC:
                s = st[pv]
                if pv % 2 == 0:
                    s["spss"] = spp.tile([PT, W], F32, name="spss")
                    st[pv]["g_spss"] = s["spss"]
                else:
                    s["spss"] = st[(pv // 2) * 2]["g_spss"]
                smm(pv, s["pj"], s["spss"])
                if pv % 2 == 1:
                    a = softmax_group(pv // 2, s["spss"])
                    st[(pv // 2) * 2]["g_attn"] = a
            b = sc - LAG_BC
            if 0 <= b < NSC:
                attn = st[(b // 2) * 2]["g_attn"]
                st[b]["bc16"] = bc_stage(b, attn)
            if sc < NSC:
                s = st[sc]
                s["vts"] = v_mms(sc, s["t16"])
                s["v16"] = v_copies(sc, s["vts"])
            if 0 <= b < NSC:
                st[b]["ut"] = ut_stage(b, st[b]["v16"], st[b]["bc16"])
                red_stage(b, st[b]["ut"])
            if sc == (NSC // 2) + LAG_RD:
                # queries 0..127 fully reduced; start first output half
                outproj_half(0)
        outproj_half(1)

    lp.__exit__(None, None, None)
    nc.compile()
    return nc


_PROGRAM = None


def _get_program():
    global _PROGRAM
    if _PROGRAM is None:
        _PROGRAM = build_program()
    return _PROGRAM


def _feature_perm():
    """dh-major shuffle: tile j, partition p  <-  head p%8, dh 16*j + p//8."""
    perm = np.empty(D, dtype=np.int64)
    for j in range(KD):
        p = np.arange(PT)
        perm[j * PT : (j + 1) * PT] = (p % H) * DH + 16 * j + p // H
    return perm


def prep_inputs(src, tgt, tgt_padding_mask, in_proj_weight, in_proj_bias,
                out_proj_weight, out_proj_bias):
    f32 = np.float32
    f16 = np.float16
    e4 = ml_dtypes.float8_e4m3
    src2 = np.asarray(src, dtype=f32).reshape(BS, D)
    tgt2 = np.asarray(tgt, dtype=f32).reshape(BS * T, D)
    mask2 = np.asarray(tgt_padding_mask).astype(bool).reshape(BS, T)
    wm = np.asarray(in_proj_weight, dtype=f32)
    wo = np.asarray(out_proj_weight, dtype=f32)

    perm = _feature_perm()
    # [in, out] layouts with permuted output features (k/q/v) and permuted
    # input rows (wo).
    wsT_f = ((wm[:D] / np.sqrt(DH)).T)[:, perm]          # [D, D]
    wtk_f = (wm[D : 2 * D].T)[:, perm] * SW              # [D, D] scaled
    wtv_f = (wm[2 * D :].T)[:, perm]                     # [D, D]
    woT_f = (wo.T)[perm, :]                              # [D, D]

    def tile128(a):  # [D, M] -> [128, KD*M], row j*128+p -> [p, j, :]
        Dm, M = a.shape
        return np.ascontiguousarray(
            a.reshape(KD, PT, M).transpose(1, 0, 2).reshape(PT, KD * M)
        )

    wsT_h = tile128(wsT_f).astype(f16)
    wtv_h = tile128(wtv_f).astype(f16)
    woT_h = tile128(woT_f).astype(f16)
    # wtk: [p, (g, jj, m)] with row (2g+jj)*128+p
    wtk_h = np.ascontiguousarray(
        np.clip(wtk_f, -224, 224)
        .reshape(2, 2, PT, D)
        .transpose(2, 0, 1, 3)
        .reshape(PT, 4 * D)
    ).astype(e4)

    em_h = np.zeros((PT, 64), dtype=f16)
    p = np.arange(PT)
    em_h[p, p % H] = 1.0
    fm_h = np.zeros((PT, PT), dtype=f16)
    for c in range(2):
        for s in range(H):
            fm_h[64 * c + s, np.arange(s, PT, H)] = 1.0

    in_maps = []
    for core in range(N_CORES):
        rows = slice(core * R, (core + 1) * R)
        kvrows = slice(core * RT, (core + 1) * RT)
        mask_c = mask2[rows]                     # [R, T]
        novalid = mask_c.all(axis=-1)            # [R]
        # score-bias row: -1e4 on masked (and not fully-masked) keys; the
        # smm accumulates 100*bmask so masked logits go to -1e6 -> exp == 0.
        bm = np.where(mask_c & ~novalid[:, None], f16(-10000.0), f16(0.0))
        # packed valid/(zero-fully-masked) factor, same layout as rec tiles
        zmp = np.empty((PT, NSC // 2, QS), dtype=f32)
        vq = (~novalid).astype(f32).reshape(NSC, QS)
        for g2 in range(NSC // 2):
            for c in range(2):
                zmp[64 * c : 64 * c + 64, g2, :] = vq[2 * g2 + c][None, :]
        tgt_c = tgt2[kvrows]                     # [RT, D]
        tg8_h = np.ascontiguousarray(
            np.clip(tgt_c.T * SX, -224, 224)
        ).astype(e4)
        tg16_h = np.ascontiguousarray(tgt_c.T).astype(f16)
        in_maps.append({
            "srcT": tile128(np.ascontiguousarray(src2[rows].T)).astype(f16),
            "tg8": tg8_h,
            "tg16": tg16_h,
            "wsT": wsT_h, "wtk": wtk_h, "wtv": wtv_h, "woT": woT_h,
            "em64": em_h, "fmrep": fm_h,
            "bmask": np.ascontiguousarray(bm.reshape(1, RT)),
            "bhun": np.concatenate([np.full((1, 8), 100.0, dtype=f16),
                                    np.zeros((1, 56), dtype=f16)], axis=1),
            "zmask": np.ascontiguousarray(zmp.reshape(PT, -1)),
        })
    return in_maps


def _numpy_fallback(src, tgt, tgt_padding_mask, in_proj_weight, in_proj_bias,
                    out_proj_weight, out_proj_bias):
    """Reference-equivalent numpy path (only for nonzero-bias inputs, which
    the benchmark never produces)."""
    B, S, _ = src.shape
    w_src, w_tgt = in_proj_weight[:D], in_proj_weight[D:]
    b_src, b_tgt = in_proj_bias[:D], in_proj_bias[D:]
    q = src @ w_src.T + b_src
    kv = tgt @ w_tgt.T + b_tgt
    k, v = kv[..., :D], kv[..., D:]
    inv = tgt_padding_mask.astype(bool)
    noval = inv.all(-1)
    inv = inv & ~noval[..., None]
    q = q.reshape(B, S, H, DH)
    k = k.reshape(B, S, T, H, DH)
    v = v.reshape(B, S, T, H, DH)
    att = np.einsum("bshd,bsthd->bhst", q, k)
    att = np.where(inv[:, None], -np.inf, att) / np.sqrt(DH)
    att = att - att.max(-1, keepdims=True)
    att = np.exp(att)
    att = att / att.sum(-1, keepdims=True)
    out = np.einsum("bhst,bsthd->bshd", att, v).reshape(B, S, D)
    out = out @ out_proj_weight.T + out_proj_bias
    return np.where(noval[..., None], 0.0, out).astype(np.float32)


def run(inputs, trace=False):
    """Returns (full_output [4,512,512] f32, BassKernelResults)."""
    in_maps = prep_inputs(**inputs)
    nc = _get_program()
    res = bass_utils.run_bass_kernel_spmd(
        nc, in_maps, core_ids=list(range(N_CORES)), trace=trace
    )
    out = np.empty((BS, D), dtype=np.float32)
    for c in range(N_CORES):
        out[c * R : (c + 1) * R] = res.results[c]["outT"].T
    return out.reshape(4, 512, D), res


def kernel(**inputs):
    inputs = {k: np.asarray(v) for k, v in inputs.items()}
    if (np.any(inputs["in_proj_bias"]) or np.any(inputs["out_proj_bias"])):
        return _numpy_fallback(**inputs)
    out, _ = run(inputs)
    return out


# revision 17
# speedup vs baseline: 1.2418x; 1.2418x over previous
# Trainium2 Bass kernel for KNN-style sparse cross-attention (v2).
#
# reference semantics:
#   q  = src @ w_src.T                           [B,S,D]
#   kv = tgt @ w_tgt.T                           [B,S,T,2D]
#   attn[b,h,s,t] = <q[b,s,h], k[b,s,t,h]> / sqrt(dh)  (per-query keys)
#   softmax over t (padding mask; fully-masked queries output 0)
#   out = (attn @ v) @ out_proj.T
#
# v2 strategy (vs v1):
#  - shard B*S = 2048 queries across 8 cores (256 queries, 8192 kv rows each)
#  - k-projection runs in fp8-e4m3 DoubleRow matmuls (2x PE throughput);
#    v-projection stays fp16 (v errors pass straight to the output).
#    k is scaled by SX*SW on chip; the descale rides the softmax exp scale.
#  - "dh-major" feature shuffle: k/q/v features are permuted so that within
#    each 128-partition tile, partition p belongs to head p%8.  The score
#    one-hot (em) and the attn broadcast (fm) then become j-independent:
#    one stationary serves all 4 feature tiles, and the attn->128-partition
#    broadcast is a single 8-partition matmul per chunk.
#  - scores for 4 chunks are packed into one PSUM bank at partition offsets
#    {0,32,64,96} via matmul tile_position, so softmax (exp/mask/sum/recip/
#    normalize) runs on 128 full partitions instead of 8.
#  - engine assignment: PE kv-proj+scores+broadcast, DVE q*k multiply (from
#    PSUM) + softmax + t-reduce, Scalar v/bc copies + exp, GPSIMD attn*v.
import os
from contextlib import ExitStack

import numpy as np
import ml_dtypes

import concourse.bacc as bacc
import concourse.mybir as mybir
import concourse.tile as tile
from concourse import bass_utils

N_CORES = 8
D = 512          # d_model
H = 8            # heads
DH = 64          # head dim
T = 32           # KNN set size per query
BS = 2048        # B*S total queries
R = BS // N_CORES     # 256 queries per core
RT = R * T            # 8192 kv rows per core
PT = 128              # partition tile
KD = D // PT          # 4 feature tiles
W = 512               # kv cols per superchunk
NSC = RT // W         # 16 superchunks
QS = W // T           # 16 queries per superchunk
SX = 16.0             # fp8 scale on tgt
SW = 512.0            # fp8 scale on w_k

F32 = mybir.dt.float32
F16 = mybir.dt.float16
F8 = mybir.dt.float8e4
AX = mybir.AxisListType
ALU = mybir.AluOpType
ACTF = mybir.ActivationFunctionType
PM = mybir.MatmulPerfMode


def build_program(n_cores=N_CORES):
    nc = bacc.Bacc(
        "TRN2",
        target_bir_lowering=False,
        debug=False,
        enable_asserts=False,
        num_devices=n_cores,
    )

    # All weight/const tensors are pre-arranged host-side into their exact
    # on-chip [128, free] layouts.
    srcT = nc.dram_tensor("srcT", [PT, KD * R], F16, kind="ExternalInput").ap()
    tg8 = nc.dram_tensor("tg8", [D, RT], F8, kind="ExternalInput").ap()
    tg16 = nc.dram_tensor("tg16", [D, RT], F16, kind="ExternalInput").ap()
    wsT = nc.dram_tensor("wsT", [PT, KD * D], F16, kind="ExternalInput").ap()
    wtk = nc.dram_tensor("wtk", [PT, 4 * D], F8, kind="ExternalInput").ap()
    wtv = nc.dram_tensor("wtv", [PT, KD * D], F16, kind="ExternalInput").ap()
    woT = nc.dram_tensor("woT", [PT, KD * D], F16, kind="ExternalInput").ap()
    em64 = nc.dram_tensor("em64", [PT, 64], F16, kind="ExternalInput").ap()
    fmrep = nc.dram_tensor("fmrep", [PT, PT], F16, kind="ExternalInput").ap()
    mask01 = nc.dram_tensor("mask01", [PT, (NSC // 2) * W], F16, kind="ExternalInput").ap()
    zmask = nc.dram_tensor("zmask", [PT, R], F32, kind="ExternalInput").ap()
    outT = nc.dram_tensor("outT", [D, R], F32, kind="ExternalOutput").ap()

    lp = nc.allow_low_precision("fp16 stores of fp32 internal math")
    lp.__enter__()
    with tile.TileContext(nc) as tc, ExitStack() as ctx:
        consts = ctx.enter_context(tc.tile_pool(name="consts", bufs=1))
        tg8p = ctx.enter_context(tc.tile_pool(name="tg8p", bufs=2))
        tg16p = ctx.enter_context(tc.tile_pool(name="tg16p", bufs=2))
        pjp = ctx.enter_context(tc.tile_pool(name="pjp", bufs=3))
        v16p = ctx.enter_context(tc.tile_pool(name="v16p", bufs=7))
        bc16p = ctx.enter_context(tc.tile_pool(name="bc16p", bufs=3))
        utp = ctx.enter_context(tc.tile_pool(name="utp", bufs=3))
        work = ctx.enter_context(tc.tile_pool(name="work", bufs=2))
        one = ctx.enter_context(tc.tile_pool(name="one", bufs=1))
        kvps = ctx.enter_context(tc.tile_pool(name="kvps", bufs=2, space="PSUM"))
        spp = ctx.enter_context(tc.tile_pool(name="spp", bufs=2, space="PSUM"))
        bcps = ctx.enter_context(tc.tile_pool(name="bcps", bufs=2, space="PSUM"))

        # ---- consts (critical-path order: qproj needs ws+src, first k-mms
        # need wtk + tg8(0); everything else can trail) ----
        ws_sb = consts.tile([PT, KD * D], F16, name="ws_sb")
        nc.sync.dma_start(ws_sb, wsT)
        src_sb = consts.tile([PT, KD * R], F16, name="src_sb")
        nc.sync.dma_start(src_sb, srcT)
        wtk_sb = consts.tile([PT, 4 * D], F8, name="wtk_sb")
        nc.sync.dma_start(wtk_sb, wtk)
        em_sb = consts.tile([PT, 64], F16, name="em_sb")
        nc.sync.dma_start(em_sb, em64)
        wtv_sb = consts.tile([PT, KD * D], F16, name="wtv_sb")
        nc.sync.dma_start(wtv_sb, wtv)

        qT = one.tile([PT, KD * R], F16, name="qT")
        oav = one.tile([PT, KD * R], F16, name="oav")

        tg8d = tg8.rearrange("(j p) n -> p j n", p=PT)
        tg16d = tg16.rearrange("(j p) n -> p j n", p=PT)

        def load_tg(sc):
            t8 = tg8p.tile([PT, KD * W], F8, name="t8")
            nc.sync.dma_start(
                t8.rearrange("p (j n) -> p j n", j=KD),
                tg8d[:, :, sc * W : (sc + 1) * W],
            )
            t16 = tg16p.tile([PT, KD * W], F16, name="t16")
            nc.sync.dma_start(
                t16.rearrange("p (j n) -> p j n", j=KD),
                tg16d[:, :, sc * W : (sc + 1) * W],
            )
            return t8, t16

        def qproj():
            for e in range(KD):
                qp = bcps.tile([PT, R], F32, name="qp", tag="bcp")
                for j in range(KD):
                    nc.tensor.matmul(
                        qp,
                        ws_sb[:, j * D + e * PT : j * D + (e + 1) * PT],
                        src_sb[:, j * R : (j + 1) * R],
                        start=(j == 0),
                        stop=(j == KD - 1),
                    )
                nc.scalar.copy(qT[:, e * R : (e + 1) * R], qp)

        # non-critical consts (after tg(0) is queued below)
        fm_sb = consts.tile([PT, PT], F16, name="fm_sb")
        mk_sb = consts.tile([PT, (NSC // 2) * W], F16, name="mk_sb")
        wo_sb = consts.tile([PT, KD * D], F16, name="wo_sb")
        zm_sb = consts.tile([PT, R], F32, name="zm_sb")

        def late_consts():
            nc.sync.dma_start(fm_sb, fmrep)
            nc.sync.dma_start(mk_sb, mask01)
            nc.sync.dma_start(wo_sb, woT)
            nc.sync.dma_start(zm_sb, zmask)

        # per-stage state, keyed by sc
        st = {}

        def k_mms(sc, t8):
            """fp8 DoubleRow k-projection: two [128, 2*W] psum tiles."""
            t8r = t8.rearrange("p (j n) -> p j n", j=KD)
            wk = wtk_sb.rearrange("p (g jj m) -> p g jj m", g=2, jj=2)
            kts = []
            for half in range(2):
                kt = kvps.tile([PT, 2 * W], F32, name="kt", tag="kv")
                for e01 in range(2):
                    e = 2 * half + e01
                    for g in range(2):
                        nc.tensor.matmul(
                            kt[:, e01 * W : (e01 + 1) * W],
                            wk[:, g, :, e * PT : (e + 1) * PT],
                            t8r[:, 2 * g : 2 * g + 2, :],
                            start=(g == 0),
                            stop=(g == 1),
                            perf_mode=PM.DoubleRow,
                        )
                kts.append(kt)
            return kts

        def pmul(sc, kts):
            """pj = k * q (broadcast over t), DVE reading k from PSUM."""
            pj = pjp.tile([PT, KD * W], F16, name="pj")
            for half, kt in enumerate(kts):
                nc.vector.tensor_mul(
                    pj.rearrange("p (e q t) -> p e q t", e=KD, t=T)[
                        :, 2 * half : 2 * half + 2
                    ],
                    kt.rearrange("p (e q t) -> p e q t", e=2, t=T),
                    qT.rearrange("p (e r) -> p e r", e=KD)[
                        :, 2 * half : 2 * half + 2, sc * QS : (sc + 1) * QS
                    ]
                    .unsqueeze(3)
                    .broadcast_to([PT, 2, QS, T]),
                )
            return pj

        def smm(sc, pj, spss):
            c = sc % 2
            for jf in range(KD):
                nc.tensor.matmul(
                    spss[64 * c : 64 * c + 64, :],
                    em_sb,
                    pj[:, jf * W : (jf + 1) * W],
                    start=(jf == 0),
                    stop=(jf == KD - 1),
                )

        def v_mms(sc, t16):
            t16r = t16.rearrange("p (j n) -> p j n", j=KD)
            vts = []
            for half in range(2):
                vt = kvps.tile([PT, 2 * W], F32, name="vt", tag="kv")
                for e01 in range(2):
                    e = 2 * half + e01
                    for j in range(KD):
                        nc.tensor.matmul(
                            vt[:, e01 * W : (e01 + 1) * W],
                            wtv_sb[:, j * D + e * PT : j * D + (e + 1) * PT],
                            t16r[:, j, :],
                            start=(j == 0),
                            stop=(j == KD - 1),
                        )
                vts.append(vt)
            return vts

        def v_copies(sc, vts):
            v16 = v16p.tile([PT, KD * W], F16, name="v16")
            for half, vt in enumerate(vts):
                nc.scalar.copy(v16[:, 2 * half * W : (2 * half + 2) * W], vt)
            return v16

        def softmax_group(g2, spss):
            exf = work.tile([PT, W], F16, name="exf")
            nc.scalar.activation(exf, spss, ACTF.Exp, scale=1.0 / (SX * SW))
            exfm = work.tile([PT, W], F16, name="exfm")
            nc.vector.tensor_mul(exfm, exf, mk_sb[:, g2 * W : (g2 + 1) * W])
            sums = work.tile([PT, QS], F32, name="sums")
            nc.vector.reduce_sum(
                sums, exfm.rearrange("p (q t) -> p q t", t=T), axis=AX.X
            )
            rec = work.tile([PT, QS], F32, name="rec")
            nc.vector.reciprocal(rec, sums)
            attn = work.tile([PT, W], F16, name="attn")
            nc.gpsimd.tensor_mul(
                attn.rearrange("p (q t) -> p q t", t=T),
                exfm.rearrange("p (q t) -> p q t", t=T),
                rec.unsqueeze(2).broadcast_to([PT, QS, T]),
            )
            return attn

        def bc_stage(sc, attn):
            c = sc % 2
            bcp = bcps.tile([PT, W], F32, name="bcp", tag="bcp")
            nc.tensor.matmul(
                bcp,
                fm_sb[64 * c : 64 * c + 8, :],
                attn[64 * c : 64 * c + 8, :],
                start=True,
                stop=True,
            )
            bc16 = bc16p.tile([PT, W], F16, name="bc16")
            nc.scalar.copy(bc16, bcp)
            return bc16

        def ut_stage(sc, v16, bc16):
            ut = utp.tile([PT, KD * W], F16, name="ut")
            nc.vector.tensor_mul(
                ut.rearrange("p (j n) -> p j n", j=KD),
                v16.rearrange("p (j n) -> p j n", j=KD),
                bc16.unsqueeze(1).broadcast_to([PT, KD, W]),
            )
            return ut

        def red_stage(sc, ut):
            nc.vector.reduce_sum(
                oav.rearrange("p (j r) -> p j r", j=KD)[
                    :, :, sc * QS : (sc + 1) * QS
                ],
                ut.rearrange("p (j q t) -> p j q t", j=KD, t=T),
                axis=AX.X,
            )

        # ---- software pipeline ----
        t8, t16 = load_tg(0)
        st[0] = dict(t8=t8, t16=t16)
        qproj()
        late_consts()

        def outproj_half(h2):
            q0, q1 = h2 * (R // 2), (h2 + 1) * (R // 2)
            for e in range(KD):
                op = bcps.tile([PT, R // 2], F32, name="op", tag="bcp")
                for j in range(KD):
                    nc.tensor.matmul(
                        op,
                        wo_sb[:, j * D + e * PT : j * D + (e + 1) * PT],
                        oav[:, j * R + q0 : j * R + q1],
                        start=(j == 0),
                        stop=(j == KD - 1),
                    )
                res = work.tile([PT, R // 2], F32, name="res")
                nc.vector.tensor_mul(res, op, zm_sb[:, q0:q1])
                nc.sync.dma_start(outT[e * PT : (e + 1) * PT, q0:q1], res)

        LAG_BC = 3   # bc/ut for sc-3
        LAG_RD = 3   # reduce right after ut (same engine, in order)
        for it in range(NSC + LAG_RD + 1):
            sc = it
            if sc < NSC:
                if sc + 1 < NSC:
                    t8n, t16n = load_tg(sc + 1)
                    st[sc + 1] = dict(t8=t8n, t16=t16n)
                s = st[sc]
                s["kts"] = k_mms(sc, s["t8"])
                s["pj"] = pmul(sc, s["kts"])
            # smm for sc-1 (k/pj of sc-1 are done; PE does not stall on DVE)
            pv = sc - 1
            if 0 <= pv < NSC:
                s = st[pv]
                if pv % 2 == 0:
                    s["spss"] = spp.tile([PT, W], F32, name="spss")
                    st[pv]["g_spss"] = s["spss"]
                else:
                    s["spss"] = st[(pv // 2) * 2]["g_spss"]
                smm(pv, s["pj"], s["spss"])
                if pv % 2 == 1:
                    a = softmax_group(pv // 2, s["spss"])
                    st[(pv // 2) * 2]["g_attn"] = a
            b = sc - LAG_BC
            if 0 <= b < NSC:
                attn = st[(b // 2) * 2]["g_attn"]
                st[b]["bc16"] = bc_stage(b, attn)
            if sc < NSC:
                s = st[sc]
                s["vts"] = v_mms(sc, s["t16"])
                s["v16"] = v_copies(sc, s["vts"])
            if 0 <= b < NSC:
                st[b]["ut"] = ut_stage(b, st[b]["v16"], st[b]["bc16"])
                red_stage(b, st[b]["ut"])
            if sc == (NSC // 2) + LAG_RD:
                # queries 0..127 fully reduced; start first output half
                outproj_half(0)
        outproj_half(1)

    lp.__exit__(None, None, None)
    nc.compile()
    return nc


_PROGRAM = None


def _get_program():
    global _PROGRAM
    if _PROGRAM is None:
        _PROGRAM = build_program()
    return _PROGRAM


def _feature_perm():
    """dh-major shuffle: tile j, partition p  <-  head p%8, dh 16*j + p//8."""
    perm = np.empty(D, dtype=np.int64)
    for j in range(KD):
        p = np.arange(PT)
        perm[j * PT : (j + 1) * PT] = (p % H) * DH + 16 * j + p // H
    return perm


def prep_inputs(src, tgt, tgt_padding_mask, in_proj_weight, in_proj_bias,
                out_proj_weight, out_proj_bias):
    f32 = np.float32
    f16 = np.float16
    e4 = ml_dtypes.float8_e4m3
    src2 = np.asarray(src, dtype=f32).reshape(BS, D)
    tgt2 = np.asarray(tgt, dtype=f32).reshape(BS * T, D)
    mask2 = np.asarray(tgt_padding_mask).astype(bool).reshape(BS, T)
    wm = np.asarray(in_proj_weight, dtype=f32)
    wo = np.asarray(out_proj_weight, dtype=f32)

    perm = _feature_perm()
    # [in, out] layouts with permuted output features (k/q/v) and permuted
    # input rows (wo).
    wsT_f = ((wm[:D] / np.sqrt(DH)).T)[:, perm]          # [D, D]
    wtk_f = (wm[D : 2 * D].T)[:, perm] * SW              # [D, D] scaled
    wtv_f = (wm[2 * D :].T)[:, perm]                     # [D, D]
    woT_f = (wo.T)[perm, :]                              # [D, D]

    def tile128(a):  # [D, M] -> [128, KD*M], row j*128+p -> [p, j, :]
        Dm, M = a.shape
        return np.ascontiguousarray(
            a.reshape(KD, PT, M).transpose(1, 0, 2).reshape(PT, KD * M)
        )

    wsT_h = tile128(wsT_f).astype(f16)
    wtv_h = tile128(wtv_f).astype(f16)
    woT_h = tile128(woT_f).astype(f16)
    # wtk: [p, (g, jj, m)] with row (2g+jj)*128+p
    wtk_h = np.ascontiguousarray(
        np.clip(wtk_f, -224, 224)
        .reshape(2, 2, PT, D)
        .transpose(2, 0, 1, 3)
        .reshape(PT, 4 * D)
    ).astype(e4)

    em_h = np.zeros((PT, 64), dtype=f16)
    p = np.arange(PT)
    em_h[p, p % H] = 1.0
    fm_h = np.zeros((PT, PT), dtype=f16)
    for c in range(2):
        for s in range(H):
            fm_h[64 * c + s, np.arange(s, PT, H)] = 1.0

    in_maps = []
    for core in range(N_CORES):
        rows = slice(core * R, (core + 1) * R)
        kvrows = slice(core * RT, (core + 1) * RT)
        mask_c = mask2[rows]                     # [R, T]
        novalid = mask_c.all(axis=-1)            # [R]
        m01 = (~(mask_c & ~novalid[:, None])).astype(f16)   # [R, T]
        # packed mask: group g2 covers sc = 2*g2, 2*g2+1; chunk c at rows 64c.
        mk = np.empty((PT, NSC // 2, W), dtype=f16)
        m01r = m01.reshape(NSC, QS, T)           # [sc, q, t]
        for g2 in range(NSC // 2):
            for c in range(2):
                mk[64 * c : 64 * c + 64, g2, :] = (
                    m01r[2 * g2 + c].reshape(W)[None, :]
                )
        tgt_c = tgt2[kvrows]                     # [RT, D]
        tg8_h = np.ascontiguousarray(
            np.clip(tgt_c.T * SX, -224, 224)
        ).astype(e4)
        tg16_h = np.ascontiguousarray(tgt_c.T).astype(f16)
        in_maps.append({
            "srcT": tile128(np.ascontiguousarray(src2[rows].T)).astype(f16),
            "tg8": tg8_h,
            "tg16": tg16_h,
            "wsT": wsT_h, "wtk": wtk_h, "wtv": wtv_h, "woT": woT_h,
            "em64": em_h, "fmrep": fm_h,
            "mask01": np.ascontiguousarray(mk.reshape(PT, -1)),
            "zmask": np.ascontiguousarray(
                np.broadcast_to((~novalid).astype(f32), (PT, R))
            ),
        })
    return in_maps


def _numpy_fallback(src, tgt, tgt_padding_mask, in_proj_weight, in_proj_bias,
                    out_proj_weight, out_proj_bias):
    """Reference-equivalent numpy path (only for nonzero-bias inputs, which
    the benchmark never produces)."""
    B, S, _ = src.shape
    w_src, w_tgt = in_proj_weight[:D], in_proj_weight[D:]
    b_src, b_tgt = in_proj_bias[:D], in_proj_bias[D:]
    q = src @ w_src.T + b_src
    kv = tgt @ w_tgt.T + b_tgt
    k, v = kv[..., :D], kv[..., D:]
    inv = tgt_padding_mask.astype(bool)
    noval = inv.all(-1)
    inv = inv & ~noval[..., None]
    q = q.reshape(B, S, H, DH)
    k = k.reshape(B, S, T, H, DH)
    v = v.reshape(B, S, T, H, DH)
    att = np.einsum("bshd,bsthd->bhst", q, k)
    att = np.where(inv[:, None], -np.inf, att) / np.sqrt(DH)
    att = att - att.max(-1, keepdims=True)
    att = np.exp(att)
    att = att / att.sum(-1, keepdims=True)
    out = np.einsum("bhst,bsthd->bshd", att, v).reshape(B, S, D)
    out = out @ out_proj_weight.T + out_proj_bias
    return np.where(noval[..., None], 0.0, out).astype(np.float32)


def run(inputs, trace=False):
    """Returns (full_output [4,512,512] f32, BassKernelResults)."""
    in_maps = prep_inputs(**inputs)
    nc = _get_program()
    res = bass_utils.run_bass_kernel_spmd(
        nc, in_maps, core_ids=list(range(N_CORES)), trace=trace
    )
    out = np.empty((BS, D), dtype=np.float32)
    for c in range(N_CORES):
        out[c * R : (c + 1) * R] = res.results[c]["outT"].T
    return out.reshape(4, 512, D), res


def kernel(**inputs):
    inputs = {k: np.asarray(v) for k, v in inputs.items()}
    if (np.any(inputs["in_proj_bias"]) or np.any(inputs["out_proj_bias"])):
        return _numpy_fallback(**inputs)
    out, _ = run(inputs)
    return out
